# revision 1
# baseline (speedup 1.0000x reference)
"""Trainium2 Bass kernel for nn_ExpertChoice (MoE routing + per-expert MLPs +
sum-weights router MLP + classification head), expert-parallel over 8 cores.

Self-contained: hardcodes full shapes (B=1024, N=E=8, D=768, K=4, NC=1000).

Per-core plan (core c == expert e):
  - router  : logits[b,:] = x[b,e,:] @ emb.T  (fp32 on PE), top-4 indices via
              DVE max8/max_index, gather token rows from x (bf16) with
              gpsimd dma_gather(transpose=True) -> selT feature-major.
  - sw MLP  : fc1 column-shard (this core's D cols of swW1), token-major out,
              PE-transpose, fc2 row-slice of swW2 -> partial logits [B,8];
              AllReduce (32KB, overlapped with expert fc1); softmax on chip.
  - experts : selT @ W1_e -> gelu -> hT (feature-major);  fc2 emits er
              token-major (lhsT = hT tiles), scaled by w[:,e] -> p.
  - combine : ReduceScatter(add) over p [B,KD] -> this core's 128-token slice
              of ws (bf16).
  - head    : PE-transpose ws slice, fc1 (full chW1) -> gelu -> hhT, fc2
              (full chW2) -> final logits for this core's tokens [128, 1000].
Host: concatenates the 8 token-slices -> [1024, 1000].
"""

import os
import numpy as np
import ml_dtypes

import concourse.bass as bass
import concourse.mybir as mybir
import concourse.tile as tile
from concourse import bacc
from concourse.masks import make_identity
from concourse.bass_utils import run_bass_kernel_spmd

F32 = mybir.dt.float32
BF16 = mybir.dt.bfloat16
I16 = mybir.dt.int16
U16 = mybir.dt.uint16
AF = mybir.ActivationFunctionType
ALU = mybir.AluOpType

NCORES = 8


class Cfg:
    def __init__(self, B=1024, D=768, NCLS=1000):
        self.B, self.D, self.NCLS = B, D, NCLS
        self.E = 8
        self.K = 4
        self.KD = self.K * D
        self.ND = self.E * D
        assert B % 128 == 0 and D % 128 == 0 and self.KD % 512 == 0
        self.DC = D // 128          # 128-chunks in D
        self.KC = self.KD // 128    # 128-chunks in KD
        self.NDC = self.ND // 128   # 128-chunks in ND
        self.TC = B // 128          # token chunks
        self.Bc = B // NCORES       # tokens per core after reduce-scatter
        self.NT = min(512, B)       # token free-dim tile for matmul
        self.NTC = B // self.NT
        self.GC = min(256, B)       # dma_gather chunk (hw limit < 512 idxs)
        self.NG = B // self.GC
        # sw fc1 col tiles (this core owns D cols), each <= 512
        self.SWT = [D // 2, D // 2] if D > 512 else [D]
        # head fc2 col tiles over NCLS, each <= 512
        nc_tiles = []
        rem = NCLS
        while rem > 0:
            t = min(500, rem)
            nc_tiles.append(t)
            rem -= t
        self.NCT = nc_tiles
        # W1/chW1 m-groups: groups of 4 chunks (512 cols)
        self.MG = self.KC // 4


def ceil_div(a, b):
    return (a + b - 1) // b


def build_nc(cfg: Cfg):
    c = cfg
    nc = bacc.Bacc("TRN2", target_bir_lowering=False, num_devices=NCORES)

    # ---- external I/O (per-core data differs only by in_map contents) ----
    xTe = nc.dram_tensor("xTe", [c.D, c.B], F32, kind="ExternalInput")
    embT = nc.dram_tensor("embT", [c.D, c.E], F32, kind="ExternalInput")
    xbf = nc.dram_tensor("xbf", [c.B * c.E, c.D], BF16, kind="ExternalInput")
    xfT = nc.dram_tensor("xfT", [c.ND, c.B], BF16, kind="ExternalInput")
    sw1s = nc.dram_tensor("sw1s", [c.ND, c.D], BF16, kind="ExternalInput")
    swb1s = nc.dram_tensor("swb1s", [128, c.D], F32, kind="ExternalInput")
    sw2s = nc.dram_tensor("sw2s", [c.D, c.E], BF16, kind="ExternalInput")
    swb2 = nc.dram_tensor("swb2", [128, c.E], F32, kind="ExternalInput")
    w1e = nc.dram_tensor("w1e", [c.KD, c.KD], BF16, kind="ExternalInput")
    b1e = nc.dram_tensor("b1e", [c.KD], F32, kind="ExternalInput")
    w2e = nc.dram_tensor("w2e", [c.KD, c.KD], BF16, kind="ExternalInput")
    b2e = nc.dram_tensor("b2e", [128, c.KD], F32, kind="ExternalInput")
    chw1 = nc.dram_tensor("chw1", [c.KD, c.KD], BF16, kind="ExternalInput")
    chb1 = nc.dram_tensor("chb1", [c.KD], F32, kind="ExternalInput")
    chw2 = nc.dram_tensor("chw2", [c.KD, c.NCLS], BF16, kind="ExternalInput")
    chb2 = nc.dram_tensor("chb2", [128, c.NCLS], F32, kind="ExternalInput")
    onehot = nc.dram_tensor("onehot", [128, c.E], F32, kind="ExternalInput")
    out = nc.dram_tensor("out", [c.Bc, c.NCLS], F32, kind="ExternalOutput")

    rg = [list(range(NCORES))]

    with tile.TileContext(nc) as tc:
        # ------- DRAM scratch -------
        with tc.tile_pool(name="dram", bufs=1, space="DRAM") as dram:
            idx_dram = dram.tile([c.B, c.K], I16)
            wl_in = dram.tile([c.B, c.E], F32)
            wl_out = dram.tile([c.B, c.E], F32, addr_space="Shared")
            p_dram = dram.tile([c.B, c.KD], BF16)
            ws_dram = dram.tile([c.Bc // 2, c.KD], BF16)
            ws_dram2 = dram.tile([c.Bc // 2, c.KD], BF16)

            _build_body(nc, tc, c, rg, locals())
    nc.finalize()
    return nc


def _dbg_out(nc, tc, c, out, src_ap):
    # debug epilogue: route some live data to `out` so truncated builds run
    import contextlib
    with tc.tile_pool(name="dbgo", bufs=1) as dp:
        dbg = dp.tile([c.Bc, c.NCLS], F32)
        nc.vector.memset(dbg, 0.0)
        s0, s1 = min(c.Bc, src_ap.shape[0]), min(c.NCLS, src_ap.shape[1])
        nc.vector.tensor_copy(dbg[0:s0, 0:s1], src_ap[0:s0, 0:s1])
        nc.sync.dma_start(out[:, :], dbg)


def _build_body(nc, tc, c, rg, T):
    xTe, embT, xbf, xfT = T["xTe"], T["embT"], T["xbf"], T["xfT"]
    sw1s, swb1s, sw2s, swb2 = T["sw1s"], T["swb1s"], T["sw2s"], T["swb2"]
    w1e, b1e, w2e, b2e = T["w1e"], T["b1e"], T["w2e"], T["b2e"]
    chw1, chb1, chw2, chb2 = T["chw1"], T["chb1"], T["chw2"], T["chb2"]
    onehot, out = T["onehot"], T["out"]
    idx_dram, wl_in, wl_out = T["idx_dram"], T["wl_in"], T["wl_out"]
    p_dram, ws_dram = T["p_dram"], T["ws_dram"]
    ws_dram2 = T["ws_dram2"]

    import contextlib

    phase = int(os.environ.get("KPHASE", "9"))
    ctx = contextlib.ExitStack()
    with ctx:
        const = ctx.enter_context(tc.tile_pool(name="const", bufs=1))
        ident = const.tile([128, 128], BF16)
        make_identity(nc, ident)

        # broadcast-style biases
        b2_sb = const.tile([128, c.KD], F32)
        nc.sync.dma_start(b2_sb, b2e[:, :])
        swb1_sb = const.tile([128, c.D], F32)
        nc.sync.dma_start(swb1_sb, swb1s[:, :])
        swb2_sb = const.tile([128, c.E], F32)
        nc.sync.dma_start(swb2_sb, swb2[:, :])
        chb2_sb = const.tile([128, c.NCLS], F32)
        nc.sync.dma_start(chb2_sb, chb2[:, :])
        oh_sb = const.tile([128, c.E], F32)
        nc.sync.dma_start(oh_sb, onehot[:, :])
        # per-partition biases [128, KC]
        b1_sb = const.tile([128, c.KC], F32)
        nc.sync.dma_start(b1_sb, b1e.rearrange("(k p) -> p k", p=128))
        chb1_sb = const.tile([128, c.KC], F32)
        nc.sync.dma_start(chb1_sb, chb1.rearrange("(k p) -> p k", p=128))

        if phase <= -1:
            _dbg_out(nc, tc, c, out, b1_sb)
            return
        # ---------------- Phase 1: router (fp32) + top-k + gather ----------
        # token-chunk-major layout so each (j, nt) gather dst is contiguous
        selT = ctx.enter_context(tc.tile_pool(name="selT", bufs=1)).tile(
            [128, c.NG, c.K * c.DC, c.GC], BF16
        )

        with tc.tile_pool(name="rt", bufs=1) as rt_pool, \
             tc.tile_pool(name="rt_psum", bufs=2, space="PSUM") as rt_psum:
            xTe_sb = rt_pool.tile([128, c.DC, c.B], F32)
            nc.sync.dma_start(xTe_sb, xTe.rearrange("(kc p) b -> p kc b", p=128))
            embT_sb = rt_pool.tile([128, c.DC, c.E], F32)
            nc.sync.dma_start(embT_sb, embT.rearrange("(kc p) e -> p kc e", p=128))

            for t in range(c.TC):
                ps = rt_psum.tile([128, c.E], F32)
                for kc in range(c.DC):
                    nc.tensor.matmul(
                        ps,
                        lhsT=xTe_sb[:, kc, t * 128:(t + 1) * 128].opt(),
                        rhs=embT_sb[:, kc, :].opt(),
                        start=(kc == 0),
                        stop=(kc == c.DC - 1),
                    )
                lg = rt_pool.tile([128, c.E], F32, tag="lg", bufs=2)
                nc.scalar.activation(lg, ps, AF.Copy)
                vals = rt_pool.tile([128, 8], F32, tag="vals", bufs=2)
                nc.vector.max(out=vals, in_=lg)
                idx8 = rt_pool.tile([128, 8], U16, tag="idx8", bufs=2)
                nc.vector.max_index(out=idx8, in_max=vals, in_values=lg)
                iota = rt_pool.tile([128, c.K], I16, tag="iota", bufs=2)
                nc.gpsimd.iota(
                    iota, pattern=[[0, c.K]], base=t * 128 * c.E,
                    channel_multiplier=c.E,
                )
                idx16 = rt_pool.tile([128, c.K], I16, tag="idx16", bufs=2)
                nc.vector.tensor_add(idx16, iota, idx8[:, 0:c.K].bitcast(I16))
                nc.sync.dma_start(idx_dram[t * 128:(t + 1) * 128, :], idx16)

            if phase <= 0:
                _dbg_out(nc, tc, c, out, idx16)
                return
            if phase == 1 and os.environ.get("KSUB") == "idx":
                idxw_dbg = None
            # rearrange indices into the gpsimd-wrapped [16, B//16] layout,
            # replicated across the 8 q7 cores (128 partitions).
            from concourse import library_config
            nc.gpsimd.load_library(library_config.mlp)
            S = c.B // 16
            for j in range(c.K):
                idxw = rt_pool.tile([128, S], I16, tag="idxw", bufs=4)
                col = idx_dram.rearrange("(s p) j -> j p s", p=16)[j]
                for r in range(8):
                    nc.sync.dma_start(idxw[16 * r:16 * r + 16, :], col)
                if os.environ.get("KSUB") == "idx":
                    if j == c.K - 1:
                        _dbg_out(nc, tc, c, out, idxw)
                        return
                    continue
                for g in range(c.NG):
                    nc.gpsimd.dma_gather(
                        out_ap=selT[:, g, c.DC * j:c.DC * (j + 1), :],
                        in_ap=xbf[:, :],
                        idxs_ap=idxw[:, g * c.GC // 16:(g + 1) * c.GC // 16],
                        num_idxs=c.GC,
                        num_idxs_reg=c.GC,
                        elem_size=c.D,
                        transpose=True,
                    )

        if phase <= 1:
            _dbg_out(nc, tc, c, out, selT[:, 0, 0, :])
            return
        # ---------------- Phase 2: sum-weights MLP ------------------------
        # fc1 token-major: out[b, m] for this core's D columns
        swh_sb = ctx.enter_context(tc.tile_pool(name="swh", bufs=1)).tile(
            [128, c.TC, c.D], BF16
        )
        with tc.tile_pool(name="swf1", bufs=3) as swf1_pool, \
             tc.tile_pool(name="swf1_psum", bufs=1, space="PSUM") as swf1_psum:
            col0 = 0
            for nt_cols in c.SWT:
                psums = []
                for t in range(c.TC):
                    psums.append(swf1_psum.tile([128, nt_cols], F32, name=f"swps{t}"))
                for kc in range(c.NDC):
                    xp = swf1_pool.tile([128, c.B], BF16, tag="xfTp", bufs=6)
                    nc.sync.dma_start(
                        xp, xfT[kc * 128:(kc + 1) * 128, :]
                    )
                    wp = swf1_pool.tile([128, nt_cols], BF16, tag="sw1p", bufs=4)
                    nc.sync.dma_start(
                        wp, sw1s[kc * 128:(kc + 1) * 128, col0:col0 + nt_cols]
                    )
                    for t in range(c.TC):
                        nc.tensor.matmul(
                            psums[t],
                            lhsT=xp[:, t * 128:(t + 1) * 128],
                            rhs=wp,
                            start=(kc == 0),
                            stop=(kc == c.NDC - 1),
                        )
                for t in range(c.TC):
                    tmp = swf1_pool.tile([128, nt_cols], F32, tag="swtmp")
                    nc.vector.tensor_add(
                        tmp, psums[t],
                        swb1_sb[:, col0:col0 + nt_cols],
                    )
                    nc.scalar.activation(
                        swh_sb[:, t, col0:col0 + nt_cols].opt(), tmp, AF.Gelu
                    )
                col0 += nt_cols

        # transpose swh -> feature-major swhT [128, DC, B]
        swhT = ctx.enter_context(tc.tile_pool(name="swhT", bufs=1)).tile(
            [128, c.DC, c.B], BF16
        )
        with tc.tile_pool(name="swt_psum", bufs=4, space="PSUM") as swt_psum, \
             tc.tile_pool(name="swf2_psum", bufs=2, space="PSUM") as swf2_psum, \
             tc.tile_pool(name="swmisc", bufs=3) as swmisc:
            for t in range(c.TC):
                for cc in range(c.DC):
                    tp = swt_psum.tile([128, 128], BF16, name="swtp")
                    nc.tensor.transpose(
                        tp, swh_sb[:, t, cc * 128:(cc + 1) * 128].opt(), ident
                    )
                    nc.scalar.activation(
                        swhT[:, cc, t * 128:(t + 1) * 128].opt(), tp, AF.Copy
                    )
            # fc2: partial logits, token-major [B, E]
            sw2_sb = swmisc.tile([128, c.DC, c.E], BF16, bufs=1)
            nc.sync.dma_start(sw2_sb, sw2s.rearrange("(kc p) e -> p kc e", p=128))
            for t in range(c.TC):
                ps = swf2_psum.tile([128, c.E], F32, name="swf2p")
                for kc in range(c.DC):
                    nc.tensor.matmul(
                        ps,
                        lhsT=swhT[:, kc, t * 128:(t + 1) * 128].opt(),
                        rhs=sw2_sb[:, kc, :].opt(),
                        start=(kc == 0),
                        stop=(kc == c.DC - 1),
                    )
                wl_sb = swmisc.tile([128, c.E], F32, tag="wl")
                nc.scalar.activation(wl_sb, ps, AF.Copy)
                nc.sync.dma_start(wl_in[t * 128:(t + 1) * 128, :], wl_sb)

        # AllReduce the sum-weights partial logits (tiny, overlaps fc1)
        nc.gpsimd.collective_compute(
            "AllReduce", ALU.add, replica_groups=rg,
            ins=[wl_in.opt()], outs=[wl_out.opt()],
        )

        if phase <= 2:
            _dbg_out(nc, tc, c, out, swhT[:, 0, :])
            return
        # ---------------- Phase 3: expert fc1 ------------------------------
        hT = ctx.enter_context(tc.tile_pool(name="hT", bufs=1)).tile(
            [128, c.KC, c.B], BF16
        )
        with tc.tile_pool(name="w1blk", bufs=2) as w1_pool, \
             tc.tile_pool(name="f1_psum", bufs=4, space="PSUM") as f1_psum:
            for mg in range(c.MG):
                blk = w1_pool.tile([128, c.KC, 512], BF16, tag="w1b")
                nc.sync.dma_start(
                    blk,
                    w1e[:, mg * 512:(mg + 1) * 512].rearrange(
                        "(kc p) m -> p kc m", p=128
                    ),
                )
                for mc in range(4):
                    m = mg * 4 + mc
                    for g in range(c.NG):
                        ps = f1_psum.tile([128, c.GC], F32, name="f1ps")
                        for kc in range(c.KC):
                            nc.tensor.matmul(
                                ps,
                                lhsT=blk[:, kc, mc * 128:(mc + 1) * 128].opt(),
                                rhs=selT[:, g, kc, :].opt(),
                                start=(kc == 0),
                                stop=(kc == c.KC - 1),
                            )
                        nc.scalar.activation(
                            hT[:, m, g * c.GC:(g + 1) * c.GC].opt(),
                            ps, AF.Gelu, bias=b1_sb[:, m:m + 1],
                        )

        if phase <= 3:
            _dbg_out(nc, tc, c, out, hT[:, 0, :])
            return
        # ---------------- softmax of routing weights (after AllReduce) -----
        # w_col[t] = softmax(wl + swb2)[:, e]  as per-partition scalars
        wcol_pool = ctx.enter_context(tc.tile_pool(name="wcol", bufs=1))
        w_col = wcol_pool.tile([128, c.TC], F32)
        with tc.tile_pool(name="smx", bufs=2) as smx:
            for t in range(c.TC):
                wlf = smx.tile([128, c.E], F32, tag="wlf")
                nc.sync.dma_start(wlf, wl_out[t * 128:(t + 1) * 128, :])
                nc.vector.tensor_add(wlf, wlf, swb2_sb)
                mx = smx.tile([128, 1], F32, tag="mx")
                nc.vector.reduce_max(out=mx, in_=wlf, axis=mybir.AxisListType.X)
                nmx = smx.tile([128, 1], F32, tag="nmx")
                nc.vector.tensor_scalar_mul(nmx, mx, -1.0)
                ex = smx.tile([128, c.E], F32, tag="ex")
                sm = smx.tile([128, 1], F32, tag="sm")
                nc.scalar.activation(ex, wlf, AF.Exp, bias=nmx, accum_out=sm)
                rs = smx.tile([128, 1], F32, tag="rs")
                nc.vector.reciprocal(rs, sm)
                # pick this expert's column via onehot + row-sum, then scale
                sel = smx.tile([128, c.E], F32, tag="sel")
                nc.vector.tensor_mul(sel, ex, oh_sb)
                num = smx.tile([128, 1], F32, tag="num")
                nc.vector.reduce_sum(out=num, in_=sel, axis=mybir.AxisListType.X)
                nc.vector.tensor_tensor(
                    out=w_col[:, t:t + 1], in0=num, in1=rs, op=ALU.mult
                )

        # ---------------- Phase 4: expert fc2 (token-major) + scale -------
        with tc.tile_pool(name="w2p", bufs=3) as w2_pool, \
             tc.tile_pool(name="f2_psum", bufs=1, space="PSUM") as f2_psum, \
             tc.tile_pool(name="pout", bufs=3) as p_pool:
            HT = c.TC // 2 if c.TC > 1 else 1
            NHALF = 2 if c.TC > 1 else 1
            for half in range(NHALF):
                for ncol in range(c.KD // 512):
                    psums = []
                    for ti in range(HT):
                        psums.append(
                            f2_psum.tile([128, 512], F32, name=f"f2ps{ti}")
                        )
                    for kc in range(c.KC):
                        wp = w2_pool.tile([128, 512], BF16, tag="w2p", bufs=4)
                        nc.sync.dma_start(
                            wp, w2e[kc * 128:(kc + 1) * 128,
                                    ncol * 512:(ncol + 1) * 512]
                        )
                        for ti in range(HT):
                            t = half * HT + ti
                            nc.tensor.matmul(
                                psums[ti],
                                lhsT=hT[:, kc, t * 128:(t + 1) * 128].opt(),
                                rhs=wp,
                                start=(kc == 0),
                                stop=(kc == c.KC - 1),
                            )
                    for ti in range(HT):
                        t = half * HT + ti
                        er = p_pool.tile([128, 512], F32, tag="er")
                        nc.vector.tensor_add(
                            er, psums[ti],
                            b2_sb[:, ncol * 512:(ncol + 1) * 512],
                        )
                        pb = p_pool.tile([128, 512], BF16, tag="pb")
                        nc.vector.tensor_scalar_mul(pb, er, w_col[:, t:t + 1])
                        nc.sync.dma_start(
                            p_dram[t * 128:(t + 1) * 128,
                                   ncol * 512:(ncol + 1) * 512],
                            pb,
                        )
                if NHALF == 2:
                    hrows = c.B // 2
                    dst = ws_dram if half == 0 else ws_dram2
                    nc.gpsimd.collective_compute(
                        "ReduceScatter", ALU.add, replica_groups=rg,
                        ins=[p_dram[half * hrows:(half + 1) * hrows, :].opt()],
                        outs=[dst.opt()],
                    )

        if phase <= 4:
            _dbg_out(nc, tc, c, out, w_col)
            return
        # ---------------- Phase 5: reduce-scatter over tokens --------------
        if c.TC == 1:
            nc.gpsimd.collective_compute(
                "ReduceScatter", ALU.add, replica_groups=rg,
                ins=[p_dram[0:c.B // 2, :].opt()], outs=[ws_dram.opt()],
            )
            nc.gpsimd.collective_compute(
                "ReduceScatter", ALU.add, replica_groups=rg,
                ins=[p_dram[c.B // 2:c.B, :].opt()], outs=[ws_dram2.opt()],
            )

        if phase <= 5:
            with tc.tile_pool(name="dbgw", bufs=1) as dw:
                wsf = dw.tile([c.Bc // 2, c.KD], BF16)
                nc.sync.dma_start(wsf, ws_dram[:, :])
                _dbg_out(nc, tc, c, out, wsf)
            return
        # ---------------- Phase 6: head on this core's Bc tokens -----------
        wsT = ctx.enter_context(tc.tile_pool(name="wsT", bufs=1)).tile(
            [128, c.KC, c.Bc], BF16
        )
        with tc.tile_pool(name="wst", bufs=2) as wst_pool, \
             tc.tile_pool(name="wst_psum", bufs=4, space="PSUM") as wst_psum:
            hb = c.Bc // 2
            ws_sb = wst_pool.tile([c.Bc, c.KD], BF16, bufs=1)
            nc.sync.dma_start(ws_sb[0:hb, :], ws_dram[:, :])
            nc.sync.dma_start(ws_sb[hb:c.Bc, :], ws_dram2[:, :])
            for kc in range(c.KC):
                tp = wst_psum.tile([128, c.Bc], BF16, name="wstp")
                nc.tensor.transpose(
                    tp, ws_sb[:, kc * 128:(kc + 1) * 128], ident[0:c.Bc, 0:c.Bc]
                )
                nc.scalar.activation(wsT[:, kc, :].opt(), tp, AF.Copy)

        hhT = ctx.enter_context(tc.tile_pool(name="hhT", bufs=1)).tile(
            [128, c.KC, c.Bc], BF16
        )
        with tc.tile_pool(name="ch1blk", bufs=2) as ch1_pool, \
             tc.tile_pool(name="h1_psum", bufs=4, space="PSUM") as h1_psum:
            for mg in range(c.MG):
                blk = ch1_pool.tile([128, c.KC, 512], BF16, tag="ch1b")
                nc.sync.dma_start(
                    blk,
                    chw1[:, mg * 512:(mg + 1) * 512].rearrange(
                        "(kc p) m -> p kc m", p=128
                    ),
                )
                for mc in range(4):
                    m = mg * 4 + mc
                    ps = h1_psum.tile([128, c.Bc], F32, name="h1ps")
                    for kc in range(c.KC):
                        nc.tensor.matmul(
                            ps,
                            lhsT=blk[:, kc, mc * 128:(mc + 1) * 128].opt(),
                            rhs=wsT[:, kc, :].opt(),
                            start=(kc == 0),
                            stop=(kc == c.KC - 1),
                        )
                    nc.scalar.activation(
                        hhT[:, m, :].opt(), ps, AF.Gelu, bias=chb1_sb[:, m:m + 1]
                    )

        with tc.tile_pool(name="ch2", bufs=3) as ch2_pool, \
             tc.tile_pool(name="h2_psum", bufs=2, space="PSUM") as h2_psum, \
             tc.tile_pool(name="osb", bufs=1) as osb_pool:
            out_sb = osb_pool.tile([c.Bc, c.NCLS], F32)
            col0 = 0
            for ncols in c.NCT:
                ps = h2_psum.tile([c.Bc, ncols], F32, name="h2ps")
                for kc in range(c.KC):
                    wp = ch2_pool.tile([128, ncols], BF16, tag="ch2p")
                    nc.sync.dma_start(
                        wp, chw2[kc * 128:(kc + 1) * 128, col0:col0 + ncols]
                    )
                    nc.tensor.matmul(
                        ps,
                        lhsT=hhT[:, kc, :].opt(),
                        rhs=wp,
                        start=(kc == 0),
                        stop=(kc == c.KC - 1),
                    )
                nc.vector.tensor_add(
                    out_sb[:, col0:col0 + ncols], ps,
                    chb2_sb[0:c.Bc, col0:col0 + ncols],
                )
                col0 += ncols
            nc.sync.dma_start(out[:, :], out_sb)


# ======================= host side =======================================

def prep_in_maps(inputs, cfg: Cfg):
    c = cfg
    bf = ml_dtypes.bfloat16
    x = np.asarray(inputs["x"], np.float32)
    emb = np.asarray(inputs["emb"], np.float32)
    W1 = np.asarray(inputs["W1"], np.float32)
    b1 = np.asarray(inputs["b1"], np.float32)
    W2 = np.asarray(inputs["W2"], np.float32)
    b2 = np.asarray(inputs["b2"], np.float32)
    swW1 = np.asarray(inputs["swW1"], np.float32)
    swb1 = np.asarray(inputs["swb1"], np.float32)
    swW2 = np.asarray(inputs["swW2"], np.float32)
    swb2 = np.asarray(inputs["swb2"], np.float32)
    chW1 = np.asarray(inputs["chW1"], np.float32)
    chb1 = np.asarray(inputs["chb1"], np.float32)
    chW2 = np.asarray(inputs["chW2"], np.float32)
    chb2 = np.asarray(inputs["chb2"], np.float32)

    embT = np.ascontiguousarray(emb.T)
    xbf = np.ascontiguousarray(x.reshape(c.B * c.E, c.D)).astype(bf)
    xfT = np.ascontiguousarray(x.reshape(c.B, c.ND).T).astype(bf)
    chw1_b = chW1.astype(bf)
    chw2_b = chW2.astype(bf)
    swb2_r = np.ascontiguousarray(np.broadcast_to(swb2.reshape(1, c.E), (128, c.E)))
    chb2_r = np.ascontiguousarray(np.broadcast_to(chb2.reshape(1, c.NCLS), (128, c.NCLS)))

    in_maps = []
    for e in range(NCORES):
        oh = np.zeros((128, c.E), np.float32)
        oh[:, e] = 1.0
        m = {
            "xTe": np.ascontiguousarray(x[:, e, :].T),
            "embT": embT,
            "xbf": xbf,
            "xfT": xfT,
            "sw1s": np.ascontiguousarray(swW1[:, e * c.D:(e + 1) * c.D]).astype(bf),
            "swb1s": np.ascontiguousarray(np.broadcast_to(swb1[e * c.D:(e + 1) * c.D].reshape(1, c.D), (128, c.D))),
            "sw2s": np.ascontiguousarray(swW2[e * c.D:(e + 1) * c.D, :]).astype(bf),
            "swb2": swb2_r,
            "w1e": W1[e].astype(bf),
            "b1e": b1[e],
            "w2e": W2[e].astype(bf),
            "b2e": np.ascontiguousarray(np.broadcast_to(b2[e].reshape(1, c.KD), (128, c.KD))),
            "chw1": chw1_b,
            "chb1": chb1,
            "chw2": chw2_b,
            "chb2": chb2_r,
            "onehot": oh,
        }
        in_maps.append(m)
    return in_maps


_CACHE = {}


def kernel(**inputs) -> np.ndarray:
    cfg = Cfg()
    key = "nc" + os.environ.get("KPHASE", "9")
    if key not in _CACHE:
        _CACHE[key] = build_nc(cfg)
    nc = _CACHE[key]
    in_maps = prep_in_maps(inputs, cfg)
    res = run_bass_kernel_spmd(
        nc, in_maps, core_ids=list(range(NCORES)),
        trace=bool(int(os.environ.get("KBENCH_TRACE", "0"))),
    )
    _CACHE["last_results"] = res
    hb = cfg.Bc // 2
    outp = np.empty((cfg.B, cfg.NCLS), np.float32)
    for cix in range(NCORES):
        o = res.results[cix]["out"]
        outp[cix * hb:(cix + 1) * hb] = o[0:hb]
        outp[cfg.B // 2 + cix * hb:cfg.B // 2 + (cix + 1) * hb] = o[hb:]
    return outp



# revision 17
# speedup vs baseline: 1.4040x; 1.4040x over previous
"""Trainium2 Bass kernel for nn_ExpertChoice (MoE routing + per-expert MLPs +
sum-weights router MLP + classification head), expert-parallel over 8 cores.

fp8 (e4m3) 3-term compensated DoubleRow matmuls on every heavy GEMM:
  operand v is split v = hi + lo (both e4m3; weights get a per-column or
  per-tensor scale folded into the epilogue).  Per 128-row k-chunk pair the
  product x@W is computed as
     MM1: DoubleRow over (xhi[2m], xhi[2m+1]) x (Whi[2m], Whi[2m+1])
     MM2: DoubleRow over (xhi[kc], xlo[kc])  x (Wlo[kc], Whi[kc])   (per kc)
  = 1.5 matmul-rows per 256 contraction rows vs 2.0 for bf16, with ~bf16
  accuracy (validated: ~0.1% per GEMM, 0.37% end-to-end).

Per-core plan (core c == expert e):
  - router  : fp32 logits on PE, top-4 via DVE max8/max_index, gpsimd
              dma_gather(transpose=True) from interleaved (hi,lo) fp8 pairs
              -> selT8 feature-major with hi/lo interleaved along tokens.
  - sw MLP  : fc1 column-shard, FEATURE-major out (no transposes), fp8;
              fc2 bf16 token-major partial logits -> AllReduce (overlapped).
  - experts : fc1 selT8 @ W1 -> gelu -> split -> hT8; fc2 (hT8 stationary,
              W2 moving) token-major, scaled by softmax weight -> p bf16.
  - combine : 2x ReduceScatter(add) over COLUMN halves of p [1024, 1536]
              -> this core's 128 tokens x 1536 features each; second RS
              overlaps the first half of head fc1 (kp-major contraction).
  - head    : fc1 kp-major fp8 -> gelu -> split; fc2 fp8 -> +bias -> out.
Host: rows c*128..(c+1)*128 of the output come from core c.
"""

import os
import numpy as np
import ml_dtypes

import concourse.bass as bass
import concourse.mybir as mybir
import concourse.tile as tile
from concourse import bacc
from concourse.masks import make_identity
from concourse.bass_utils import run_bass_kernel_spmd

F32 = mybir.dt.float32
BF16 = mybir.dt.bfloat16
FP8 = mybir.dt.float8e4
I16 = mybir.dt.int16
U16 = mybir.dt.uint16
AF = mybir.ActivationFunctionType
ALU = mybir.AluOpType
DR = mybir.MatmulPerfMode.DoubleRow

NCORES = 8
E4 = ml_dtypes.float8_e4m3


class Cfg:
    def __init__(self):
        self.B = 1024
        self.D = 768
        self.NCLS = 1000
        self.NCP = 1024          # padded classes
        self.E = 8
        self.K = 4
        self.KD = self.K * self.D            # 3072
        self.ND = self.E * self.D            # 6144
        self.DC = self.D // 128              # 6
        self.KC = self.KD // 128             # 24
        self.KP = self.KC // 2               # 12 chunk pairs
        self.NDC = self.ND // 128            # 48
        self.NDP = self.NDC // 2             # 24 pairs
        self.TC = self.B // 128              # 8 token chunks
        self.Bc = self.B // NCORES           # 128 tokens per core for head
        self.GC = 256                        # gather chunk (hw limit < 512)
        self.NG = self.B // self.GC          # 4
        self.MGW = 256                       # w1-as-stationary m-group width
        self.NMG = self.KD // self.MGW       # 12


def build_nc(cfg: Cfg):
    c = cfg
    nc = bacc.Bacc("TRN2", target_bir_lowering=False, num_devices=NCORES)

    xTe = nc.dram_tensor("xTe", [c.D, c.B], F32, kind="ExternalInput")
    embT = nc.dram_tensor("embT", [c.D, c.E], F32, kind="ExternalInput")
    # gather source: token rows, interleaved (hi,lo) per feature = 1536 B
    xg8 = nc.dram_tensor("xg8", [c.B * c.E, 2 * c.D], FP8, kind="ExternalInput")
    # sw fc1 moving: row (kp*128+p), cols (kc2, tok, hilo)
    xsw8 = nc.dram_tensor("xsw8", [c.NDP * 128, 2 * c.B * 2], FP8,
                          kind="ExternalInput")
    # sw fc1 stationary: row (kp*128+p), cols (kc2, pl, 768)
    sw18 = nc.dram_tensor("sw18", [c.NDP * 128, 2 * 2 * c.D], FP8,
                          kind="ExternalInput")
    swb1s = nc.dram_tensor("swb1s", [c.D], F32, kind="ExternalInput")
    swsi = nc.dram_tensor("swsi", [c.D], F32, kind="ExternalInput")
    sw2s = nc.dram_tensor("sw2s", [c.D, c.E], BF16, kind="ExternalInput")
    swb2 = nc.dram_tensor("swb2", [128, c.E], F32, kind="ExternalInput")
    onehot = nc.dram_tensor("onehot", [128, c.E], F32, kind="ExternalInput")
    # expert fc1 stationary: row (mg*KD + kc*128 + p), cols (pl, 256)
    w1e8 = nc.dram_tensor("w1e8", [c.NMG * c.KD, 2 * c.MGW], FP8,
                          kind="ExternalInput")
    b1e = nc.dram_tensor("b1e", [c.KD], F32, kind="ExternalInput")
    si1 = nc.dram_tensor("si1", [c.KD], F32, kind="ExternalInput")
    # expert fc2 moving: row (kp*256 + p*2 + kc), cols (nb, pl, 512)
    w2e8 = nc.dram_tensor("w2e8", [c.KP * 256, (c.KD // 512) * 2 * 512], FP8,
                          kind="ExternalInput")
    b2s = nc.dram_tensor("b2s", [128, c.KD], F32, kind="ExternalInput")
    # head fc1 stationary: row (mg*KD + kc*128 + p), cols (pl, 256)
    ch18 = nc.dram_tensor("ch18", [c.NMG * c.KD, 2 * c.MGW], FP8,
                          kind="ExternalInput")
    chb1 = nc.dram_tensor("chb1", [c.KD], F32, kind="ExternalInput")
    sic1 = nc.dram_tensor("sic1", [c.KD], F32, kind="ExternalInput")
    # head fc2 moving: row (kp*256 + p*2 + kc), cols (nb, pl, 512)
    ch28 = nc.dram_tensor("ch28", [c.KP * 256, 2 * 2 * 512], FP8,
                          kind="ExternalInput")
    chb2s = nc.dram_tensor("chb2s", [128, c.NCP], F32, kind="ExternalInput")
    out = nc.dram_tensor("out", [c.Bc, c.NCLS], F32, kind="ExternalOutput")

    rg = [list(range(NCORES))]

    with tile.TileContext(nc) as tc:
        with tc.tile_pool(name="dram", bufs=1, space="DRAM") as dram:
            idx_dram = dram.tile([c.B, c.K], I16)
            wl_in = dram.tile([c.B, c.E], F32)
            wl_out = dram.tile([c.B, c.E], F32, addr_space="Shared")
            p_a = dram.tile([c.B, 2048], BF16)
            p_b = dram.tile([c.B, 1024], BF16)
            ws_a = dram.tile([c.Bc, 2048], BF16)
            ws_b = dram.tile([c.Bc, 1024], BF16)
            _body(nc, tc, c, rg, locals())
    nc.finalize()
    return nc


def _dbg_out(nc, tc, c, out, src_ap):
    with tc.tile_pool(name="dbgo", bufs=1) as dp:
        dbg = dp.tile([c.Bc, c.NCLS], F32)
        nc.vector.memset(dbg, 0.0)
        s0 = min(c.Bc, src_ap.shape[0])
        s1 = min(c.NCLS, src_ap.free_size())
        nc.vector.tensor_copy(dbg[0:s0, 0:s1], src_ap[0:s0, 0:s1])
        nc.sync.dma_start(out[:, :], dbg)


def _body(nc, tc, c, rg, T):
    xTe, embT, xg8, xsw8, sw18 = T["xTe"], T["embT"], T["xg8"], T["xsw8"], T["sw18"]
    swb1s, swsi, sw2s, swb2, onehot = T["swb1s"], T["swsi"], T["sw2s"], T["swb2"], T["onehot"]
    w1e8, b1e, si1, w2e8, b2s = T["w1e8"], T["b1e"], T["si1"], T["w2e8"], T["b2s"]
    ch18, chb1, sic1, ch28, chb2s = T["ch18"], T["chb1"], T["sic1"], T["ch28"], T["chb2s"]
    out = T["out"]
    idx_dram, wl_in, wl_out = T["idx_dram"], T["wl_in"], T["wl_out"]
    p_a, p_b, ws_a, ws_b = T["p_a"], T["p_b"], T["ws_a"], T["ws_b"]

    phase = int(os.environ.get("KPHASE", "9"))
    NB2 = c.KD // 512            # 6 fc2 col blocks
    HKC = c.KC // 2              # 12 ws chunks per column half

    _stack = []

    def _open(cm):
        obj = cm.__enter__()
        _stack.append(cm)
        return obj

    def _close(cm):
        assert _stack and _stack[-1] is cm, "pool close order"
        _stack.pop().__exit__(None, None, None)

    def _close_all():
        while _stack:
            _stack.pop().__exit__(None, None, None)

    const = _open(tc.tile_pool(name="const", bufs=1))
    ident = const.tile([128, 128], BF16)
    make_identity(nc, ident)

    # small consts on the Activation queue (SP stays free for streaming)
    swb2_sb = const.tile([128, c.E], F32)
    nc.scalar.dma_start(swb2_sb, swb2[:, :])
    oh_sb = const.tile([128, c.E], F32)
    nc.scalar.dma_start(oh_sb, onehot[:, :])
    b1_sb = const.tile([128, c.KC], F32)
    nc.scalar.dma_start(b1_sb, b1e.rearrange("(k p) -> p k", p=128))
    si1_sb = const.tile([128, c.KC], F32)
    nc.scalar.dma_start(si1_sb, si1.rearrange("(k p) -> p k", p=128))
    swb1_sb = const.tile([128, c.DC], F32)
    nc.scalar.dma_start(swb1_sb, swb1s.rearrange("(k p) -> p k", p=128))
    swsi_sb = const.tile([128, c.DC], F32)
    nc.scalar.dma_start(swsi_sb, swsi.rearrange("(k p) -> p k", p=128))
    chb1_sb = const.tile([128, c.KC], F32)
    nc.scalar.dma_start(chb1_sb, chb1.rearrange("(k p) -> p k", p=128))
    sic1_sb = const.tile([128, c.KC], F32)
    nc.scalar.dma_start(sic1_sb, sic1.rearrange("(k p) -> p k", p=128))
    b2_sb = const.tile([128, c.KD], F32)
    nc.scalar.dma_start(b2_sb, b2s[:, :])
    chb2_sb = const.tile([128, c.NCP], F32)
    nc.scalar.dma_start(chb2_sb, chb2s[:, :])
    sw2_sb = const.tile([128, c.DC, c.E], BF16)
    nc.scalar.dma_start(sw2_sb, sw2s.rearrange("(kc p) e -> p kc e", p=128))

    # hT8 outlives selT8 -> open first (LIFO pool discipline)
    hT8 = _open(tc.tile_pool(name="hT8", bufs=1)).tile(
        [128, c.KC, 2, c.B], FP8
    )

    # ------------- Phase 1: router + top-k + gather ------------------------
    selT8_cm = tc.tile_pool(name="selT", bufs=1)
    selT8 = _open(selT8_cm).tile([128, c.NG, c.K, c.DC, c.GC, 2], FP8)

    with tc.tile_pool(name="rt", bufs=1) as rt_pool, \
         tc.tile_pool(name="rt_psum", bufs=2, space="PSUM") as rt_psum:
        xTe_sb = rt_pool.tile([128, c.DC, c.B], F32)
        xTe_r = xTe.rearrange("(kc p) b -> p kc b", p=128)
        nc.sync.dma_start(xTe_sb[:, :, 0:512], xTe_r[:, :, 0:512])
        nc.sync.dma_start(xTe_sb[:, :, 512:1024], xTe_r[:, :, 512:1024])
        embT_sb = rt_pool.tile([128, c.DC, c.E], F32)
        nc.sync.dma_start(embT_sb, embT.rearrange("(kc p) e -> p kc e", p=128))

        for t in range(c.TC):
            ps = rt_psum.tile([128, c.E], F32)
            for kc in range(c.DC):
                nc.tensor.matmul(
                    ps,
                    lhsT=xTe_sb[:, kc, t * 128:(t + 1) * 128].opt(),
                    rhs=embT_sb[:, kc, :].opt(),
                    start=(kc == 0),
                    stop=(kc == c.DC - 1),
                )
            lg = rt_pool.tile([128, c.E], F32, tag="lg", bufs=2)
            nc.scalar.activation(lg, ps, AF.Copy)
            vals = rt_pool.tile([128, 8], F32, tag="vals", bufs=2)
            nc.vector.max(out=vals, in_=lg)
            idx8 = rt_pool.tile([128, 8], U16, tag="idx8", bufs=2)
            nc.vector.max_index(out=idx8, in_max=vals, in_values=lg)
            iota = rt_pool.tile([128, c.K], I16, tag="iota", bufs=2)
            nc.gpsimd.iota(
                iota, pattern=[[0, c.K]], base=t * 128 * c.E,
                channel_multiplier=c.E,
            )
            idx16 = rt_pool.tile([128, c.K], I16, tag="idx16", bufs=2)
            nc.vector.tensor_add(idx16, iota, idx8[:, 0:c.K].bitcast(I16))
            nc.scalar.dma_start(idx_dram[t * 128:(t + 1) * 128, :], idx16)

        from concourse import library_config
        nc.gpsimd.load_library(library_config.mlp)
        S = c.B // 16
        for j in range(c.K):
            idxw = rt_pool.tile([128, S], I16, tag="idxw", bufs=4)
            col = idx_dram.rearrange("(s p) j -> j p s", p=16)[j]
            for r in range(8):
                nc.scalar.dma_start(idxw[16 * r:16 * r + 16, :], col)
            for g in range(c.NG):
                nc.gpsimd.dma_gather(
                    out_ap=selT8[:, g, j].opt(keep_dims={0}).rearrange(
                        "p (c i) -> p c i", c=12
                    ),
                    in_ap=xg8[:, :],
                    idxs_ap=idxw[:, g * c.GC // 16:(g + 1) * c.GC // 16],
                    num_idxs=c.GC,
                    num_idxs_reg=c.GC,
                    elem_size=2 * c.D,
                    transpose=True,
                )

    if phase <= 1:
        _dbg_out(nc, tc, c, out, selT8[:, 0, 0, 0, :, 0])
        _close_all()
        return

    # ------------- Phase 2: sum-weights fc1 (fp8, feature-major) -----------
    swhT_cm = tc.tile_pool(name="swhT", bufs=1)
    swhT = _open(swhT_cm).tile([128, c.DC, c.B], BF16)

    xsw_r = xsw8.rearrange("(kp p) (kc t two) -> p kp kc t two",
                           p=128, kc=2, two=2)
    sw1_r = sw18.rearrange("(kp p) (kc pl m) -> p kp kc pl m",
                           p=128, kc=2, pl=2)
    with tc.tile_pool(name="sw1p", bufs=4) as sw1_pool, \
         tc.tile_pool(name="swxp", bufs=3) as swx_pool, \
         tc.tile_pool(name="sw_psum", bufs=1, space="PSUM") as sw_psum:
        for bt in range(2):
            psums = [
                sw_psum.tile([128, 512], F32, name=f"swps{cc}")
                for cc in range(c.DC)
            ]
            for kp in range(c.NDP):
                s1t = sw1_pool.tile([128, 2, 2, c.D], FP8, tag="s1t")
                nc.sync.dma_start(s1t, sw1_r[:, kp])
                xt = swx_pool.tile([128, 2, 512, 2], FP8, tag="swx")
                nc.sync.dma_start(
                    xt, xsw_r[:, kp, :, bt * 512:(bt + 1) * 512, :]
                )
                for cc in range(c.DC):
                    nc.tensor.matmul(
                        psums[cc],
                        lhsT=s1t[:, :, 1, cc * 128:(cc + 1) * 128],
                        rhs=xt[:, :, :, 0],
                        start=(kp == 0), stop=False,
                        perf_mode=DR,
                    )
                    for kc in range(2):
                        nc.tensor.matmul(
                            psums[cc],
                            lhsT=s1t[:, kc, :, cc * 128:(cc + 1) * 128],
                            rhs=xt[:, kc].rearrange("p t two -> p two t"),
                            start=False,
                            stop=(kp == c.NDP - 1 and kc == 1),
                            perf_mode=DR,
                        )
            for cc in range(c.DC):
                nc.scalar.activation(
                    swhT[:, cc, bt * 512:(bt + 1) * 512].opt(),
                    psums[cc], AF.Gelu,
                    bias=swb1_sb[:, cc:cc + 1],
                    scale=swsi_sb[:, cc:cc + 1],
                )

    # sw fc2 (bf16, tiny) -> partial logits -> AllReduce
    with tc.tile_pool(name="swf2", bufs=2) as swf2_pool, \
         tc.tile_pool(name="swf2_psum", bufs=2, space="PSUM") as swf2_psum:
        for t in range(c.TC):
            ps = swf2_psum.tile([128, c.E], F32, name="swf2p")
            for kc in range(c.DC):
                nc.tensor.matmul(
                    ps,
                    lhsT=swhT[:, kc, t * 128:(t + 1) * 128].opt(),
                    rhs=sw2_sb[:, kc, :].opt(),
                    start=(kc == 0),
                    stop=(kc == c.DC - 1),
                )
            wl_sb = swf2_pool.tile([128, c.E], F32, tag="wl")
            nc.scalar.activation(wl_sb, ps, AF.Copy)
            nc.scalar.dma_start(wl_in[t * 128:(t + 1) * 128, :], wl_sb)

    nc.gpsimd.collective_compute(
        "AllReduce", ALU.add, replica_groups=rg,
        ins=[wl_in.opt()], outs=[wl_out.opt()],
    )
    _close(swhT_cm)

    if phase <= 2:
        _dbg_out(nc, tc, c, out, selT8[:, 0, 0, 0, :, 0])
        _close_all()
        return

    # ------------- Phase 3: expert fc1 (fp8) -> hT8 ------------------------

    with tc.tile_pool(name="w1p", bufs=3) as w1_pool, \
         tc.tile_pool(name="f1_psum", bufs=4, space="PSUM") as f1_psum, \
         tc.tile_pool(name="f1hf", bufs=4) as f1hf:
        for mg in range(c.NMG):
            w1t = w1_pool.tile([128, c.KC, 2, c.MGW], FP8, tag="w1t")
            nc.sync.dma_start(
                w1t,
                w1e8[mg * c.KD:(mg + 1) * c.KD, :].rearrange(
                    "(kc p) (pl m) -> p kc pl m", p=128, pl=2
                ),
            )
            for mc in range(c.MGW // 128):
                m = mg * (c.MGW // 128) + mc
                for g in range(c.NG):
                    ps = f1_psum.tile([128, c.GC], F32, name="f1ps")
                    for j in range(c.K):
                        for cp in range(c.DC // 2):
                            kb = j * c.DC + 2 * cp
                            nc.tensor.matmul(
                                ps,
                                lhsT=w1t[:, kb:kb + 2, 1,
                                         mc * 128:(mc + 1) * 128],
                                rhs=selT8[:, g, j, 2 * cp:2 * cp + 2, :, 0],
                                start=(j == 0 and cp == 0), stop=False,
                                perf_mode=DR,
                            )
                            for ci in range(2):
                                cc = 2 * cp + ci
                                nc.tensor.matmul(
                                    ps,
                                    lhsT=w1t[:, j * c.DC + cc, :,
                                             mc * 128:(mc + 1) * 128],
                                    rhs=selT8[:, g, j, cc].rearrange(
                                        "p t two -> p two t"
                                    ),
                                    start=False,
                                    stop=(j == c.K - 1
                                          and cp == c.DC // 2 - 1 and ci == 1),
                                    perf_mode=DR,
                                )
                    hf = f1hf.tile([128, c.GC], F32, tag="hf")
                    nc.scalar.activation(
                        hf, ps, AF.Gelu,
                        bias=b1_sb[:, m:m + 1],
                        scale=si1_sb[:, m:m + 1],
                    )
                    nc.vector.tensor_copy(
                        hT8[:, m, 0, g * c.GC:(g + 1) * c.GC].opt(), hf
                    )
                    nc.vector.tensor_tensor(
                        out=hT8[:, m, 1, g * c.GC:(g + 1) * c.GC].opt(),
                        in0=hf,
                        in1=hT8[:, m, 0, g * c.GC:(g + 1) * c.GC].opt(),
                        op=ALU.subtract,
                    )
    _close(selT8_cm)

    if phase <= 3:
        _dbg_out(nc, tc, c, out, hT8[:, 0, 0, :])
        _close_all()
        return

    # ------------- softmax of routing weights (after AllReduce) ------------
    w_col = _open(tc.tile_pool(name="wcol", bufs=1)).tile([128, c.TC], F32)
    with tc.tile_pool(name="smx", bufs=2) as smx:
        for t in range(c.TC):
            wlf = smx.tile([128, c.E], F32, tag="wlf")
            nc.scalar.dma_start(wlf, wl_out[t * 128:(t + 1) * 128, :])
            nc.vector.tensor_add(wlf, wlf, swb2_sb)
            mx = smx.tile([128, 1], F32, tag="mx")
            nc.vector.reduce_max(out=mx, in_=wlf, axis=mybir.AxisListType.X)
            nmx = smx.tile([128, 1], F32, tag="nmx")
            nc.vector.tensor_scalar_mul(nmx, mx, -1.0)
            ex = smx.tile([128, c.E], F32, tag="ex")
            sm = smx.tile([128, 1], F32, tag="sm")
            nc.scalar.activation(ex, wlf, AF.Exp, bias=nmx, accum_out=sm)
            rs = smx.tile([128, 1], F32, tag="rs")
            nc.vector.reciprocal(rs, sm)
            sel = smx.tile([128, c.E], F32, tag="sel")
            nc.vector.tensor_mul(sel, ex, oh_sb)
            num = smx.tile([128, 1], F32, tag="num")
            nc.vector.reduce_sum(out=num, in_=sel, axis=mybir.AxisListType.X)
            nc.vector.tensor_tensor(
                out=w_col[:, t:t + 1], in0=num, in1=rs, op=ALU.mult
            )

    # ---- prefetch head weights on the Activation queue (runs during fc2) --
    CH1_BUFS, CH2_BUFS = 7, 6
    ch1_pool = _open(tc.tile_pool(name="ch1p", bufs=CH1_BUFS))
    ch1_tiles = []
    # pre-issue only the ring depth on Act (flows during fc2 without
    # blocking); later tiles are created+loaded just-in-time in the head.
    def _ch1_load(mg):
        c1t = ch1_pool.tile([128, c.KC, 2, c.MGW], FP8, tag="c1t")
        nc.scalar.dma_start(
            c1t,
            ch18[mg * c.KD:(mg + 1) * c.KD, :].rearrange(
                "(kc p) (pl m) -> p kc pl m", p=128, pl=2
            ),
        )
        return c1t

    for mg in range(CH1_BUFS):
        ch1_tiles.append(_ch1_load(mg))
    ch2_pool = _open(tc.tile_pool(name="ch2p", bufs=CH2_BUFS))
    ch2_tiles = []
    for i in range(CH2_BUFS):
        nb, kp = divmod(i, c.KP)
        c2t = ch2_pool.tile([128, 2, 2, 512], FP8, tag="c2t")
        nc.scalar.dma_start(
            c2t,
            ch28[kp * 256:(kp + 1) * 256,
                 nb * 1024:(nb + 1) * 1024].rearrange(
                "(p kc) (pl m) -> p kc pl m", kc=2, pl=2
            ),
        )
        ch2_tiles.append(c2t)

    # ------------- Phase 4: expert fc2 (fp8) + scale -> p ------------------
    w2_r = None
    with tc.tile_pool(name="w2p", bufs=8) as w2_pool, \
         tc.tile_pool(name="f2_psum", bufs=1, space="PSUM") as f2_psum, \
         tc.tile_pool(name="pout", bufs=4) as p_pool:
        for nb in range(NB2):
            psums = [
                f2_psum.tile([128, 512], F32, name=f"f2ps{t}")
                for t in range(c.TC)
            ]
            for kp in range(c.KP):
                w2t = w2_pool.tile([128, 2, 2, 512], FP8, tag="w2t")
                nc.sync.dma_start(
                    w2t,
                    w2e8[kp * 256:(kp + 1) * 256,
                         nb * 1024:(nb + 1) * 1024].rearrange(
                        "(p kc) (pl m) -> p kc pl m", kc=2, pl=2
                    ),
                )
                for t in range(c.TC):
                    nc.tensor.matmul(
                        psums[t],
                        lhsT=hT8[:, 2 * kp:2 * kp + 2, 0,
                                 t * 128:(t + 1) * 128],
                        rhs=w2t[:, :, 1, :],
                        start=(kp == 0), stop=False,
                        perf_mode=DR,
                    )
                    for kc in range(2):
                        nc.tensor.matmul(
                            psums[t],
                            lhsT=hT8[:, 2 * kp + kc, :,
                                     t * 128:(t + 1) * 128],
                            rhs=w2t[:, kc],
                            start=False,
                            stop=(kp == c.KP - 1 and kc == 1),
                            perf_mode=DR,
                        )
            for t in range(c.TC):
                er = p_pool.tile([128, 512], F32, tag="er")
                nc.vector.tensor_add(
                    er, psums[t], b2_sb[:, nb * 512:(nb + 1) * 512]
                )
                pb = p_pool.tile([128, 512], BF16, tag="pb")
                nc.vector.tensor_scalar_mul(pb, er, w_col[:, t:t + 1])
                if nb < 4:
                    dst = p_a[t * 128:(t + 1) * 128,
                              nb * 512:(nb + 1) * 512]
                else:
                    dst = p_b[t * 128:(t + 1) * 128,
                              (nb - 4) * 512:(nb - 3) * 512]
                nc.scalar.dma_start(dst, pb)
            if nb == 3:
                nc.gpsimd.collective_compute(
                    "ReduceScatter", ALU.add, replica_groups=rg,
                    ins=[p_a.opt()], outs=[ws_a.opt()],
                )
        nc.gpsimd.collective_compute(
            "ReduceScatter", ALU.add, replica_groups=rg,
            ins=[p_b.opt()], outs=[ws_b.opt()],
        )
    if phase <= 4:
        _dbg_out(nc, tc, c, out, w_col)
        _close_all()
        return

    # ------------- Phase 5: head fc1 (fp8, m-group streaming) --------------
    hhT8 = _open(tc.tile_pool(name="hhT8", bufs=1)).tile(
        [128, c.KC, 2, c.Bc], FP8
    )
    with tc.tile_pool(name="wst", bufs=2) as wst_pool, \
         tc.tile_pool(name="h1_psum", bufs=4, space="PSUM") as h1_psum, \
         tc.tile_pool(name="wst_psum", bufs=3, space="PSUM") as wst_psum, \
         tc.tile_pool(name="hh_hf", bufs=4) as hh_hf:
        wsT8 = wst_pool.tile([128, c.KC, 2, c.Bc], FP8, bufs=1)
        for half, (ws_h, nchunk) in enumerate(((ws_a, 16), (ws_b, 8))):
            ws_sb = wst_pool.tile([c.Bc, nchunk * 128], BF16,
                                  tag=f"wssb{half}", bufs=1)
            nc.scalar.dma_start(ws_sb, ws_h[:, :])
            for ck in range(nchunk):
                kc = half * 16 + ck
                tp = wst_psum.tile([128, c.Bc], BF16, name="wstp")
                nc.tensor.transpose(
                    tp, ws_sb[:, ck * 128:(ck + 1) * 128], ident
                )
                nc.vector.tensor_copy(wsT8[:, kc, 0, :].opt(), tp)
                nc.vector.tensor_tensor(
                    out=wsT8[:, kc, 1, :].opt(), in0=tp,
                    in1=wsT8[:, kc, 0, :].opt(), op=ALU.subtract,
                )
        for mg in range(c.NMG):
            c1t = ch1_tiles[mg] if mg < len(ch1_tiles) else _ch1_load(mg)
            for mc in range(c.MGW // 128):
                m = mg * (c.MGW // 128) + mc
                ps = h1_psum.tile([128, c.Bc], F32, name="h1ps")
                for kp in range(c.KP):
                    nc.tensor.matmul(
                        ps,
                        lhsT=c1t[:, 2 * kp:2 * kp + 2, 1,
                                 mc * 128:(mc + 1) * 128],
                        rhs=wsT8[:, 2 * kp:2 * kp + 2, 0, :],
                        start=(kp == 0), stop=False,
                        perf_mode=DR,
                    )
                    for kc in range(2):
                        nc.tensor.matmul(
                            ps,
                            lhsT=c1t[:, 2 * kp + kc, :,
                                     mc * 128:(mc + 1) * 128],
                            rhs=wsT8[:, 2 * kp + kc],
                            start=False,
                            stop=(kp == c.KP - 1 and kc == 1),
                            perf_mode=DR,
                        )
                hf = hh_hf.tile([128, c.Bc], F32, tag="hhf")
                nc.scalar.activation(
                    hf, ps, AF.Gelu,
                    bias=chb1_sb[:, m:m + 1],
                    scale=sic1_sb[:, m:m + 1],
                )
                nc.vector.tensor_copy(hhT8[:, m, 0, :].opt(), hf)
                nc.vector.tensor_tensor(
                    out=hhT8[:, m, 1, :].opt(), in0=hf,
                    in1=hhT8[:, m, 0, :].opt(), op=ALU.subtract,
                )

    if phase <= 5:
        _dbg_out(nc, tc, c, out, hhT8[:, 0, 0, :])
        _close_all()
        return

    # ------------- Phase 6: head fc2 -> out --------------------------------
    with tc.tile_pool(name="h2_psum", bufs=2, space="PSUM") as h2_psum, \
         tc.tile_pool(name="osb", bufs=1) as osb_pool:
        out_sb = osb_pool.tile([c.Bc, c.NCP], F32)
        for nb in range(2):
            ps = h2_psum.tile([c.Bc, 512], F32, name="h2ps")
            for kp in range(c.KP):
                i = nb * c.KP + kp
                if i < len(ch2_tiles):
                    c2t = ch2_tiles[i]
                else:
                    c2t = ch2_pool.tile([128, 2, 2, 512], FP8, tag="c2t")
                    nc.sync.dma_start(
                        c2t,
                        ch28[kp * 256:(kp + 1) * 256,
                             nb * 1024:(nb + 1) * 1024].rearrange(
                            "(p kc) (pl m) -> p kc pl m", kc=2, pl=2
                        ),
                    )
                nc.tensor.matmul(
                    ps,
                    lhsT=hhT8[:, 2 * kp:2 * kp + 2, 0, :],
                    rhs=c2t[:, :, 1, :],
                    start=(kp == 0), stop=False,
                    perf_mode=DR,
                )
                for kc in range(2):
                    nc.tensor.matmul(
                        ps,
                        lhsT=hhT8[:, 2 * kp + kc, :, :],
                        rhs=c2t[:, kc],
                        start=False,
                        stop=(kp == c.KP - 1 and kc == 1),
                        perf_mode=DR,
                    )
            nc.vector.tensor_add(
                out_sb[:, nb * 512:(nb + 1) * 512], ps,
                chb2_sb[0:c.Bc, nb * 512:(nb + 1) * 512],
            )
        nc.sync.dma_start(out[:, :], out_sb[:, 0:c.NCLS])

    _close_all()


# ======================= host side =======================================

def _split8(a):
    hi = np.asarray(a, np.float32).astype(E4)
    lo = (np.asarray(a, np.float32) - hi.astype(np.float32)).astype(E4)
    return hi, lo


def _wsplit_cols(W, clip=224.0):
    """per-column scale -> (hi, lo, sinv[cols])"""
    s = clip / (np.abs(W).max(axis=0) + 1e-30)
    hi, lo = _split8(W * s[None, :])
    return hi, lo, (1.0 / s).astype(np.float32)


def _wsplit_tensor(W, clip=224.0):
    s = clip / (np.abs(W).max() + 1e-30)
    hi, lo = _split8(W * s)
    return hi, lo, np.float32(1.0 / s)


def _pack_moving_rows(hi, lo):
    """[K, M] hi/lo -> rows (kp*256 + p*2 + kc), cols (nb, pl, 512)."""
    K, M = hi.shape
    KP = K // 256
    NB = M // 512
    outp = np.empty((KP, 128, 2, NB, 2, 512), E4)
    h = hi.reshape(KP, 2, 128, NB, 512)
    l = lo.reshape(KP, 2, 128, NB, 512)
    outp[:, :, :, :, 0, :] = l.transpose(0, 2, 1, 3, 4)
    outp[:, :, :, :, 1, :] = h.transpose(0, 2, 1, 3, 4)
    return np.ascontiguousarray(outp.reshape(KP * 256, NB * 2 * 512))


def prep_in_maps(inputs, cfg: Cfg):
    c = cfg
    bf = ml_dtypes.bfloat16
    x = np.asarray(inputs["x"], np.float32)
    emb = np.asarray(inputs["emb"], np.float32)
    W1 = np.asarray(inputs["W1"], np.float32)
    b1 = np.asarray(inputs["b1"], np.float32)
    W2 = np.asarray(inputs["W2"], np.float32)
    b2 = np.asarray(inputs["b2"], np.float32)
    swW1 = np.asarray(inputs["swW1"], np.float32)
    swb1 = np.asarray(inputs["swb1"], np.float32)
    swW2 = np.asarray(inputs["swW2"], np.float32)
    swb2v = np.asarray(inputs["swb2"], np.float32)
    chW1 = np.asarray(inputs["chW1"], np.float32)
    chb1v = np.asarray(inputs["chb1"], np.float32)
    chW2 = np.asarray(inputs["chW2"], np.float32)
    chb2v = np.asarray(inputs["chb2"], np.float32)

    embT = np.ascontiguousarray(emb.T)

    # gather source: interleaved (hi, lo) per feature
    xr = x.reshape(c.B * c.E, c.D)
    xhi, xlo = _split8(xr)
    xg8 = np.empty((c.B * c.E, c.D, 2), E4)
    xg8[:, :, 0] = xhi
    xg8[:, :, 1] = xlo
    xg8 = xg8.reshape(c.B * c.E, 2 * c.D)

    # sw fc1 moving: xf^T [ND, B] -> rows (kp*128+p), cols (kc, tok, 2)
    xfT = np.ascontiguousarray(x.reshape(c.B, c.ND).T)
    fhi, flo = _split8(xfT)
    xsw = np.empty((c.NDP, 2, 128, c.B, 2), E4)
    xsw[:, :, :, :, 0] = fhi.reshape(c.NDP, 2, 128, c.B)
    xsw[:, :, :, :, 1] = flo.reshape(c.NDP, 2, 128, c.B)
    xsw8 = np.ascontiguousarray(
        xsw.transpose(0, 2, 1, 3, 4).reshape(c.NDP * 128, 2 * c.B * 2)
    )

    # head fc1 stationary (shared): rows (mg, k), cols (pl, 256)
    c1hi, c1lo, sic1v = _wsplit_cols(chW1)
    chp = np.empty((c.NMG, c.KD, 2, c.MGW), E4)
    for mg in range(c.NMG):
        chp[mg, :, 0, :] = c1lo[:, mg * c.MGW:(mg + 1) * c.MGW]
        chp[mg, :, 1, :] = c1hi[:, mg * c.MGW:(mg + 1) * c.MGW]
    ch18 = np.ascontiguousarray(chp.reshape(c.NMG * c.KD, 2 * c.MGW))

    # head fc2 moving (shared): per-tensor scale, padded to 1024 cols
    c2hi, c2lo, sic2 = _wsplit_tensor(chW2)
    c2hi_p = np.zeros((c.KD, c.NCP), E4)
    c2lo_p = np.zeros((c.KD, c.NCP), E4)
    c2hi_p[:, 0:c.NCLS] = c2hi
    c2lo_p[:, 0:c.NCLS] = c2lo
    ch28 = _pack_moving_rows(c2hi_p, c2lo_p)
    # out_sb = psum + chb2/sic2 on-chip; host multiplies by sic2
    chb2_r = np.zeros((128, c.NCP), np.float32)
    chb2_r[:, 0:c.NCLS] = (chb2v / sic2)[None, :]

    b2_r = np.broadcast_to(b2.reshape(c.E, 1, c.KD), (c.E, 128, c.KD))

    in_maps = []
    si2s = []
    for e in range(NCORES):
        w1hi, w1lo, si1v = _wsplit_cols(W1[e])
        w1p = np.empty((c.NMG, c.KD, 2, c.MGW), E4)
        for mg in range(c.NMG):
            w1p[mg, :, 0, :] = w1lo[:, mg * c.MGW:(mg + 1) * c.MGW]
            w1p[mg, :, 1, :] = w1hi[:, mg * c.MGW:(mg + 1) * c.MGW]
        w1e8 = np.ascontiguousarray(w1p.reshape(c.NMG * c.KD, 2 * c.MGW))

        w2hi, w2lo, si2 = _wsplit_tensor(W2[e])
        w2e8 = _pack_moving_rows(w2hi, w2lo)
        si2s.append(si2)

        sw1c = swW1[:, e * c.D:(e + 1) * c.D]
        s1hi, s1lo, siswv = _wsplit_cols(sw1c)
        swp = np.empty((c.NDP, 2, 128, 2, c.D), E4)
        swp[:, :, :, 0, :] = s1lo.reshape(c.NDP, 2, 128, c.D)
        swp[:, :, :, 1, :] = s1hi.reshape(c.NDP, 2, 128, c.D)
        sw18 = np.ascontiguousarray(
            swp.transpose(0, 2, 1, 3, 4).reshape(c.NDP * 128, 2 * 2 * c.D)
        )

        # p = (psum + b2/si2) * (softmax * si2) == (true_er + b2) * softmax
        oh = np.zeros((128, c.E), np.float32)
        oh[:, e] = si2

        m = {
            "xTe": np.ascontiguousarray(x[:, e, :].T),
            "embT": embT,
            "xg8": xg8,
            "xsw8": xsw8,
            "sw18": sw18,
            "swb1s": np.ascontiguousarray(swb1[e * c.D:(e + 1) * c.D]),
            "swsi": siswv,
            "sw2s": np.ascontiguousarray(
                swW2[e * c.D:(e + 1) * c.D, :]).astype(bf),
            "swb2": np.ascontiguousarray(
                np.broadcast_to(swb2v.reshape(1, c.E), (128, c.E))),
            "onehot": oh,
            "w1e8": w1e8,
            "b1e": b1[e],
            "si1": si1v,
            "w2e8": w2e8,
            "b2s": np.ascontiguousarray(b2_r[e] / si2),
            "ch18": ch18,
            "chb1": chb1v,
            "sic1": sic1v,
            "ch28": ch28,
            "chb2s": chb2_r,
        }
        in_maps.append(m)
    return in_maps, sic2


_CACHE = {}


def kernel(**inputs) -> np.ndarray:
    cfg = Cfg()
    key = "nc" + os.environ.get("KPHASE", "9")
    if key not in _CACHE:
        _CACHE[key] = build_nc(cfg)
    nc = _CACHE[key]
    in_maps, sic2 = prep_in_maps(inputs, cfg)
    res = run_bass_kernel_spmd(
        nc, in_maps, core_ids=list(range(NCORES)),
        trace=bool(int(os.environ.get("KBENCH_TRACE", "0"))),
    )
    _CACHE["last_results"] = res
    outp = np.empty((cfg.B, cfg.NCLS), np.float32)
    for cix in range(NCORES):
        o = res.results[cix]["out"]
        outp[cix * cfg.Bc:(cix + 1) * cfg.Bc] = o * sic2
    return outp
